# revision 28
# baseline (speedup 1.0000x reference)
"""Trainium2 Bass kernel for nn_Net_83794811945603 (3-layer GraphSAGE, mean agg).

Computation (N=50000 nodes, E=800000 edges):
    h0 = x @ W_map + b_map                                  [N,128]
    h1 = relu(mean_agg(h0) @ Wl1 + bl1 + h0 @ Wr1)          [N,128]
    h2 = relu(mean_agg(h1) @ Wl2 + bl2 + h1 @ Wr2)          [N,256]
    out = log_softmax(mean_agg(h2) @ Wl3 + bl3 + h2 @ Wr3)  [N,40]
where mean_agg(h)[i] = mean over edges (s->i) of h[s].

Strategy (8 NeuronCores, SPMD), v2:
  - Nodes sharded row-wise: core c owns nodes [c*6250, (c+1)*6250) and all
    edges whose dst lands there.  Weights replicated.
  - Full previous-layer features in DRAM via chunked AllGather (two pieces:
    src rows [0,3200) and [3200,6250) per core) so the second piece's AG
    overlaps the first piece's gather+compute (SBUF f32 accumulator carries
    piece-0 partial sums).
  - Edge rows gathered via SWDGE dma_gather with <=1920-idx calls (the
    128-entry descriptor ring caps a call at 2032 idxs; 1920 = 15 chunks).
    Variable per-(piece,supertile) segment sizes: edges packed contiguously
    per dst tile with only tail padding, so ~no pad gather traffic.
  - Segment-sum by dst via TensorE one-hot matmuls per 128-edge chunk.
    Chunks may straddle two adjacent dst tiles; the second tile's one-hot
    uses an iota bank offset by 128 (dl values are dst-local relative to the
    chunk's first tile, in [0,256); bf16-exact).  One-hot built on VectorE
    in bf16 (2x DVE rate).
  - Layer 3 aggregates z3 = h2 @ Wl3 (40 valid cols, 128-wide rows: gather
    rows must be >=256B) and computes only the first 64 lanes.
  - log_softmax without max-subtraction (|logit| small), Exp accumulated
    per tile, single batched Ln across all 49 tiles -> no activation-table
    thrashing.
"""

import os
import sys

sys.path.insert(0, "/opt/trn_rl_repo")

import numpy as np
import ml_dtypes

import concourse.bass as bass
import concourse.bacc as bacc
import concourse.tile as tile
import concourse.mybir as mybir
from concourse.bass_utils import run_bass_kernel_spmd

F32 = mybir.dt.float32
BF16 = mybir.dt.bfloat16
I16 = mybir.dt.int16

N = 50000
E = 800000
F_IN = 500
F_IN_PAD = 512
D = 128            # ID_DIM == HID
D2 = 256           # 2*HID
NCLS = 40
Z3W = 64           # computed width of z3 lanes (>=NCLS, <=128)
NCORES = 8
NLOC = N // NCORES          # 6250
NT = (NLOC + 127) // 128    # 49 dst tiles per core
NLOC_PAD = NT * 128         # 6272
NSUP = (NT + 1) // 2        # 25 supertiles (2 dst tiles each, last has 1)
P0T = 25                    # piece 0 = src rows [0, 3200) = tiles 0..24
P0ROWS = P0T * 128          # 3200
P1ROWS = NLOC - P0ROWS      # 3050
TAB0 = NCORES * P0ROWS      # 25600 (< 32767, int16-indexable)
TAB1 = NCORES * P1ROWS      # 24400
ELEM = 128                  # gather row width (256B = SWDGE minimum)
MAXG = 1024                 # max idxs per call: single-packet 16KB cap = 64 rows x 16 engines
A1T = 19                    # piece-0 AG sub-chunk 1 = tiles 0..18
A1ROWS = A1T * 128          # 2432
A2ROWS = P0ROWS - A1ROWS    # 768 (small: the last chunk gates the gathers)


def _ts(i, size=128):
    return slice(i * size, (i + 1) * size)


def _cdiv(a, b):
    return (a + b - 1) // b


# ---------------- shared layout (host & program agree) ----------------

def make_layout(m):
    """m: [2, NT] int array of per-(piece, dst-tile) slot counts (>=128).

    Returns the full segment structure shared by prepare_inputs and
    build_program.  Segment = (piece p, supertile T); within a segment the
    tiles' slots are contiguous with tail padding to a 128 multiple.
    """
    segs = []
    slot_base = 0
    chunk_base = 0
    fine_base = np.zeros((2, NT), dtype=np.int64)
    dl_base_parts = []
    for p in (0, 1):
        for T in range(NSUP):
            tl = [2 * T] + ([2 * T + 1] if 2 * T + 1 < NT else [])
            tiles = []
            a = 0
            for t in tl:
                fine_base[p, t] = slot_base + a
                tiles.append((t, a, a + int(m[p, t])))
                a += int(m[p, t])
            slots = _cdiv(a, 128) * 128
            nch = slots // 128
            # structural first tile per chunk (pad region -> last tile)
            starts = [x[1] for x in tiles]
            tf = []
            for c in range(nch):
                s = c * 128
                it = 0
                for i, st in enumerate(starts):
                    if st <= s:
                        it = i
                tf.append(it)
            dl_base_parts.append(
                np.repeat([128 * tiles[it][0] for it in tf], 128))
            # builds: (sone col0, k chunks, iota offset, dl col0).  dl values
            # are relative to each chunk's own first tile, so ALL first-tile
            # one-hots build in a single batched is_equal; straddle chunks
            # get a second 128-shifted build at an extra sone column.
            builds = [(0, nch, 0, 0)]
            str_col = {}
            next_extra = nch
            for i, (t, ta, tb) in enumerate(tiles):
                if ta % 128 != 0:
                    str_col[i] = next_extra
                    builds.append((next_extra, 1, 128, ta // 128))
                    next_extra += 1
            # matmuls, chunk-major: (chunk, tile idx, sone col, start, stop)
            mm = []
            for c in range(nch):
                it = tf[c]
                mm.append([c, it, c])
                if it + 1 < len(tiles) and tiles[it + 1][1] > c * 128 \
                        and tiles[it + 1][1] < c * 128 + 128:
                    mm.append([c, it + 1, str_col[it + 1]])
            first_of = {}
            last_of = {}
            for j, (c, it, sc) in enumerate(mm):
                if it not in first_of:
                    first_of[it] = j
                last_of[it] = j
            mm = [(c, it, sc, j == first_of[it], j == last_of[it])
                  for j, (c, it, sc) in enumerate(mm)]
            # gather call pieces (chunk-aligned, <= MAXG idxs each)
            npieces = _cdiv(nch, MAXG // 128)
            base_ch = nch // npieces
            extra = nch % npieces
            pieces = []
            s0 = 0
            for i in range(npieces):
                k = (base_ch + (1 if i < extra else 0)) * 128
                pieces.append((s0, s0 + k))
                s0 += k
            segs.append(dict(p=p, T=T, slot_base=slot_base,
                             chunk_base=chunk_base, slots=slots, nch=nch,
                             tiles=tiles, builds=builds, mm=mm,
                             pieces=pieces, nstr=next_extra - nch))
            slot_base += slots
            chunk_base += nch
    return dict(
        m=m, segs=segs, SLTOT=slot_base, NCHTOT=chunk_base,
        GMAX=max(s["nch"] for s in segs),
        SMAX=max(s["nch"] + s["nstr"] for s in segs),
        dl_base=np.concatenate(dl_base_parts),
        fine_base=fine_base,
    )


# ---------------- program ----------------

def build_program(layout, build_stage=None):
    if build_stage is None:
        build_stage = int(os.environ.get("KERNEL_BUILD_STAGE", "3"))
    segs = layout["segs"]
    SLTOT = layout["SLTOT"]
    NCHTOT = layout["NCHTOT"]
    GMAX = layout["GMAX"]
    SMAX = layout["SMAX"]
    SEG = {(s["p"], s["T"]): s for s in segs}

    nc = bacc.Bacc("TRN2", target_bir_lowering=False, debug=False,
                   num_devices=NCORES, num_swdge_queues=4)

    # ---- I/O ----
    xT_d = nc.dram_tensor("xT", [F_IN_PAD, NLOC_PAD], BF16, kind="ExternalInput")
    idx_d = nc.dram_tensor("idx", [128, SLTOT // 16], I16, kind="ExternalInput")
    dl_d = nc.dram_tensor("dl", [128, NCHTOT], BF16, kind="ExternalInput")
    invdeg_d = nc.dram_tensor("invdeg", [128, NLOC_PAD], F32, kind="ExternalInput")
    iota_d = nc.dram_tensor("iota", [128, 256], BF16, kind="ExternalInput")
    ident_d = nc.dram_tensor("ident", [128, 128], BF16, kind="ExternalInput")
    ident32_d = nc.dram_tensor("ident32", [128, 128], F32, kind="ExternalInput")
    wmap_d = nc.dram_tensor("wmap", [128, F_IN_PAD], BF16, kind="ExternalInput")
    bmap_d = nc.dram_tensor("bmap", [128, 1], F32, kind="ExternalInput")
    wl1_d = nc.dram_tensor("wl1", [128, D], BF16, kind="ExternalInput")
    wr1_d = nc.dram_tensor("wr1", [128, D], BF16, kind="ExternalInput")
    bl1_d = nc.dram_tensor("bl1", [128, 1], F32, kind="ExternalInput")
    wl2_d = nc.dram_tensor("wl2", [128, D2], BF16, kind="ExternalInput")
    wr2_d = nc.dram_tensor("wr2", [128, D2], BF16, kind="ExternalInput")
    bl2_d = nc.dram_tensor("bl2", [128, 2], F32, kind="ExternalInput")
    wl3_d = nc.dram_tensor("wl3", [128, 256], BF16, kind="ExternalInput")
    wr3_d = nc.dram_tensor("wr3", [128, 256], BF16, kind="ExternalInput")
    bl3_d = nc.dram_tensor("bl3", [128, 1], F32, kind="ExternalInput")
    out_d = nc.dram_tensor("out", [NLOC, NCLS], F32, kind="ExternalOutput")

    # internal DRAM: per-piece local stores + AllGathered full tables
    h0loc = [nc.dram_tensor("h0loc_a", [P0ROWS, D], BF16),
             nc.dram_tensor("h0loc_b", [P1ROWS, D], BF16)]
    h1loc = [nc.dram_tensor("h1loc_a", [P0ROWS, D], BF16),
             nc.dram_tensor("h1loc_b", [P1ROWS, D], BF16)]
    z3loc = [nc.dram_tensor("z3loc_a", [P0ROWS, D], BF16),
             nc.dram_tensor("z3loc_b", [P1ROWS, D], BF16)]
    h0full = [nc.dram_tensor("h0full_a", [TAB0, D], BF16, addr_space="Shared"),
              nc.dram_tensor("h0full_b", [TAB1, D], BF16, addr_space="Shared")]
    h1full = [nc.dram_tensor("h1full_a", [TAB0, D], BF16, addr_space="Shared"),
              nc.dram_tensor("h1full_b", [TAB1, D], BF16, addr_space="Shared")]
    z3full = [nc.dram_tensor("z3full_a", [TAB0, D], BF16, addr_space="Shared"),
              nc.dram_tensor("z3full_b", [TAB1, D], BF16, addr_space="Shared")]

    groups = [list(range(NCORES))]
    qrot = [0]

    with tile.TileContext(nc) as tc:
        with (
            tc.tile_pool(name="const", bufs=1) as cp,
            tc.tile_pool(name="hres", bufs=1) as hp,
            tc.tile_pool(name="gat", bufs=6) as gp,
            tc.tile_pool(name="sone", bufs=4) as sp,
            tc.tile_pool(name="work", bufs=3) as wp,
            tc.tile_pool(name="xin", bufs=8) as xp,
            tc.tile_pool(name="pa", bufs=4, space="PSUM") as pa,
            tc.tile_pool(name="po", bufs=2, space="PSUM") as po,
            tc.tile_pool(name="pt", bufs=2, space="PSUM") as pt,
        ):
            # ---- load constants (stage0-critical first) ----
            ident = cp.tile([128, 128], BF16)
            wmap = cp.tile([128, F_IN_PAD], BF16)
            bmap = cp.tile([128, 1], F32)
            iota = cp.tile([128, 256], BF16)
            dl_sb = cp.tile([128, NCHTOT], BF16)
            idx_sb = cp.tile([128, SLTOT // 16], I16)
            wl1 = cp.tile([128, D], BF16)
            wr1 = cp.tile([128, D], BF16)
            bl1 = cp.tile([128, 1], F32)
            invdeg = cp.tile([128, NLOC_PAD], F32)
            wl2 = cp.tile([128, D2], BF16)
            wr2 = cp.tile([128, D2], BF16)
            bl2 = cp.tile([128, 2], F32)
            wl3 = cp.tile([128, 256], BF16)
            wr3 = cp.tile([128, 256], BF16)
            bl3 = cp.tile([128, 1], F32)
            ident32 = cp.tile([128, 128], F32)
            # stage0-critical consts only; the big tables load after the
            # x slabs are queued so they don't delay stage0
            for sb_t, dr in [(ident, ident_d), (wmap, wmap_d),
                             (bmap, bmap_d), (iota, iota_d)]:
                nc.scalar.dma_start(out=sb_t[:], in_=dr[:])

            def load_big_consts():
                for sb_t, dr in [(wl1, wl1_d), (wr1, wr1_d), (bl1, bl1_d),
                                 (wl2, wl2_d), (wr2, wr2_d), (bl2, bl2_d),
                                 (wl3, wl3_d), (wr3, wr3_d), (bl3, bl3_d),
                                 (ident32, ident32_d)]:
                    nc.scalar.dma_start(out=sb_t[:], in_=dr[:])
                nc.scalar.dma_start(out=dl_sb[:], in_=dl_d[:])
                for big_t, big_d, np_ in ((idx_sb, idx_d, 4),
                                          (invdeg, invdeg_d, 4)):
                    w = big_t.shape[1] // np_
                    for c4 in range(np_):
                        nc.scalar.dma_start(
                            out=big_t[:, c4 * w:(c4 + 1) * w],
                            in_=big_d[:, c4 * w:(c4 + 1) * w])

            # persistent SBUF
            h0T = hp.tile([128, NLOC_PAD], BF16, tag="hA")
            h1T = hp.tile([128, NLOC_PAD], BF16, tag="hB")
            accT = hp.tile([128, NLOC_PAD], BF16, tag="hD")
            logits = hp.tile([128, NT * NCLS], F32, tag="hE")
            lseA = hp.tile([128, 64], F32, tag="hF")

            def ag(src_loc, dst_full):
                nc.gpsimd.collective_compute(
                    "AllGather", mybir.AluOpType.bypass, replica_groups=groups,
                    ins=[src_loc[:]], outs=[dst_full[:]])

            def ag3(loc, full, sub):
                # piece-0 table AllGathered in two sub-chunks so each fires
                # as soon as its tiles are stored; piece-1 in one shot
                if sub == 0:
                    nc.gpsimd.collective_compute(
                        "AllGather", mybir.AluOpType.bypass,
                        replica_groups=groups,
                        ins=[loc[0][0:A1ROWS]],
                        outs=[full[0][0:NCORES * A1ROWS]])
                elif sub == 1:
                    nc.gpsimd.collective_compute(
                        "AllGather", mybir.AluOpType.bypass,
                        replica_groups=groups,
                        ins=[loc[0][A1ROWS:P0ROWS]],
                        outs=[full[0][NCORES * A1ROWS:TAB0]])
                else:
                    ag(loc[1], full[1])

            def store_loc(t, sb_tile, loc):
                if t < P0T:
                    nc.sync.dma_start(out=loc[0][_ts(t)], in_=sb_tile[0:128, :])
                else:
                    r0 = (t - P0T) * 128
                    rows = min(128, P1ROWS - r0)
                    nc.sync.dma_start(out=loc[1][r0:r0 + rows, :],
                                      in_=sb_tile[0:rows, :])

            def emit_transpose_store(t, srcT, loc, vcopy=False):
                ptr = pt.tile([128, 128], BF16, tag="pt")
                nc.tensor.transpose(ptr[:], srcT[:, _ts(t)], ident[:])
                nm = wp.tile([128, 128], BF16, tag="nm")
                if vcopy:
                    # vector copy: keeps the scalar queue (x loads) rolling
                    nc.vector.tensor_scalar(out=nm[:], in0=ptr[:],
                                            scalar1=0.0, scalar2=None,
                                            op0=mybir.AluOpType.add)
                else:
                    nc.scalar.copy(nm[:], ptr[:])
                store_loc(t, nm, loc)

            # ---- stage 0: h0T = W_map^T @ xT + b_map ----
            chunks = []
            c0 = 0
            while c0 < NLOC_PAD:
                chunks.append((c0, min(512, NLOC_PAD - c0)))
                c0 += 512
            # chunks 0-6 rotate 8 bufs on the scalar queue; chunks 7-12
            # get dedicated buffers and load up-front on the sync queue
            # (before any store can block it)
            all_slabs = {}
            for ci, (c0, cw) in enumerate(chunks):
                if ci % 2 == 0:
                    continue
                all_slabs[ci] = [xp.tile([128, 512], BF16, tag="xs2",
                                         name=f"xs{ci}_{k}", bufs=24)
                                 for k in range(4)]
                for k in range(4):
                    nc.sync.dma_start(out=all_slabs[ci][k][:, 0:cw],
                                      in_=xT_d[_ts(k), c0:c0 + cw])
            big_loaded = [False]
            for ci, (c0, cw) in enumerate(chunks):
                if ci % 2 == 0:
                    slabs = [xp.tile([128, 512], BF16, tag="xs",
                                     name=f"xs{ci}_{k}") for k in range(4)]
                    for k in range(4):
                        nc.scalar.dma_start(out=slabs[k][:, 0:cw],
                                            in_=xT_d[_ts(k), c0:c0 + cw])
                else:
                    slabs = all_slabs[ci]
                if ci == len(chunks) - 1 and not big_loaded[0]:
                    load_big_consts()
                    big_loaded[0] = True
                ps = po.tile([128, 512], F32, tag="po")
                for k in range(4):
                    nc.tensor.matmul(ps[:, 0:cw], wmap[:, _ts(k)],
                                     slabs[k][:, 0:cw],
                                     start=(k == 0), stop=(k == 3))
                nc.vector.tensor_scalar(out=h0T[:, c0:c0 + cw],
                                        in0=ps[:, 0:cw],
                                        scalar1=bmap[:, 0:1], scalar2=None,
                                        op0=mybir.AluOpType.add)
                for t in range(c0 // 128, (c0 + cw) // 128):
                    emit_transpose_store(t, h0T, h0loc, vcopy=True)
                if ci == 4:
                    ag3(h0loc, h0full, 0)
                if ci == 6:
                    ag3(h0loc, h0full, 1)
            ag3(h0loc, h0full, 2)

            # ---- generic aggregation machinery ----
            def gather_seg(seg, src_tab, g):
                for (s0, s1) in seg["pieces"]:
                    nidx = s1 - s0
                    b = seg["slot_base"] + s0
                    # multi-packet mode: single_packet coalesces each
                    # engine's stream into ONE <=16KB packet, capping a call
                    # at 64 rows/engine (1024 idxs).  Multi-packet has no cap.
                    nc.gpsimd.dma_gather(
                        g[:, s0 // 128:s1 // 128, :], src_tab[:],
                        idx_sb[:, b // 16:(b + nidx) // 16],
                        nidx, nidx, ELEM,
                        single_packet=True, queue_num=qrot[0])
                    qrot[0] = (qrot[0] + 1) % 4

            def agg_build(seg):
                """Build the segment's one-hot matrices (VectorE only)."""
                sone = sp.tile([128, SMAX, 128], BF16, tag="S")
                cb = seg["chunk_base"]
                for (col0, k, ioff, dlc0) in seg["builds"]:
                    io_b = iota[:, ioff:ioff + 128] \
                        .rearrange("p (o j) -> p o j", o=1) \
                        .broadcast_to([128, k, 128])
                    dl_b = dl_sb[:, cb + dlc0:cb + dlc0 + k] \
                        .rearrange("p (c o) -> p c o", o=1) \
                        .broadcast_to([128, k, 128])
                    nc.vector.tensor_tensor(
                        out=sone[:, col0:col0 + k, :], in0=io_b, in1=dl_b,
                        op=mybir.AluOpType.is_equal)
                return sone

            def agg_mm(seg, g, sone, pa_tiles, elem, inject=None):
                """inject: per-tile SBUF tile to seed the PSUM bank with
                (piece-0 partial sums), via an identity matmul."""
                if inject is not None:
                    for i, (t, _, _) in enumerate(seg["tiles"]):
                        nc.tensor.matmul(pa_tiles[i][0:elem, :],
                                         ident[:, 0:elem], inject[:, _ts(t)],
                                         start=True, stop=False)
                for (c, it, sc, st, sp_) in seg["mm"]:
                    nc.tensor.matmul(pa_tiles[it][0:elem, :],
                                     g[:, c, 0:elem], sone[:, sc, :],
                                     start=(st and inject is None), stop=sp_)

            def layer(src_tabs, elem, mean_dt, body, ag_locfull=None,
                      epilogue_mid=None, epilogue_mid2=None,
                      epilogue_end=None):
                # Software-pipelined piece phases: S-builds are hoisted so
                # VectorE never stalls PE's next supertile, and the
                # mean+body (or accT unload) of supertile T runs while PE
                # streams supertile T+1's aggregation matmuls.
                def run_piece(p, finish):
                    sones = {}

                    def build(T):
                        if T >= NSUP:
                            return
                        seg = SEG[(p, T)]
                        sones[T] = agg_build(seg)

                    build(0)
                    build(1)
                    pend = None
                    for T in range(NSUP):
                        seg = SEG[(p, T)]
                        g = gp.tile([128, GMAX, ELEM], BF16, tag="g")
                        gather_seg(seg, src_tabs[p], g)
                        pa_tiles = [pa.tile([128, 128], F32, tag="pa",
                                            name=f"pa{p}_{T}_{i}")
                                    for i in range(len(seg["tiles"]))]
                        agg_mm(seg, g, sones.pop(T), pa_tiles, elem,
                               inject=(accT if p == 1 else None))
                        build(T + 2)
                        if pend is not None:
                            finish(*pend)
                        pend = (SEG[(p, T)], pa_tiles)
                    finish(*pend)

                def finish0(seg, pa_tiles):
                    for i, (t, _, _) in enumerate(seg["tiles"]):
                        nc.scalar.copy(accT[0:elem, _ts(t)],
                                       pa_tiles[i][0:elem, :])

                def finish1(seg, pa_tiles):
                    for i, (t, _, _) in enumerate(seg["tiles"]):
                        mean = wp.tile([128, 128], mean_dt, tag="mean")
                        nc.vector.tensor_tensor(
                            out=mean[0:elem, :], in0=pa_tiles[i][0:elem, :],
                            in1=invdeg[0:elem, _ts(t)],
                            op=mybir.AluOpType.mult)
                        body(t, mean)
                    if seg["T"] == 9 and ag_locfull is not None:
                        ag3(*ag_locfull, 0)
                    if seg["T"] == 12:
                        if ag_locfull is not None:
                            ag3(*ag_locfull, 1)
                        if epilogue_mid is not None:
                            epilogue_mid()
                    if seg["T"] == 18 and epilogue_mid2 is not None:
                        epilogue_mid2()

                run_piece(0, finish0)
                run_piece(1, finish1)
                if ag_locfull is not None:
                    ag3(*ag_locfull, 2)
                if epilogue_end is not None:
                    epilogue_end()

            # ---- layer 1 ----
            def l1_body(t, mean):
                p1 = po.tile([128, 512], F32, tag="po")
                nc.tensor.matmul(p1[:, 0:128], wl1[:], mean[:],
                                 start=True, stop=False)
                nc.tensor.matmul(p1[:, 0:128], wr1[:], h0T[:, _ts(t)],
                                 start=False, stop=True)
                nc.scalar.activation(out=h1T[:, _ts(t)], in_=p1[:, 0:128],
                                     func=mybir.ActivationFunctionType.Relu,
                                     bias=bl1[:, 0:1], scale=1.0)
                emit_transpose_store(t, h1T, h1loc)

            if build_stage >= 1:
                layer(h0full, D, BF16, l1_body, ag_locfull=(h1loc, h1full))

            # ---- layer 2 (+ z3 projection) ----
            h2T0 = hp.tile([128, NLOC_PAD], BF16, tag="hA")  # reuses h0T slot
            h2T1 = hp.tile([128, NLOC_PAD], BF16, tag="hC")

            def l2_body(t, mean):
                for hh, h2T_h in enumerate((h2T0, h2T1)):
                    p2 = po.tile([128, 512], F32, tag="po")
                    nc.tensor.matmul(p2[:, 0:128], wl2[:, _ts(hh)], mean[:],
                                     start=True, stop=False)
                    nc.tensor.matmul(p2[:, 0:128], wr2[:, _ts(hh)],
                                     h1T[:, _ts(t)], start=False, stop=True)
                    nc.scalar.activation(
                        out=h2T_h[:, _ts(t)], in_=p2[:, 0:128],
                        func=mybir.ActivationFunctionType.Relu,
                        bias=bl2[:, hh:hh + 1], scale=1.0)
                pz = po.tile([128, 512], F32, tag="po")
                nc.tensor.matmul(pz[:, 0:128], wl3[:, 0:128],
                                 h2T0[:, _ts(t)], start=True, stop=False)
                nc.tensor.matmul(pz[:, 0:128], wl3[:, 128:256],
                                 h2T1[:, _ts(t)], start=False, stop=True)
                zt = wp.tile([128, 128], BF16, tag="zt")
                nc.scalar.copy(zt[:], pz[:, 0:128])
                ptr = pt.tile([128, 128], BF16, tag="pt")
                nc.tensor.transpose(ptr[:], zt[:], ident[:])
                nmz = wp.tile([128, 128], BF16, tag="nm")
                nc.scalar.copy(nmz[:], ptr[:])
                store_loc(t, nmz, z3loc)

            if build_stage >= 2:
                layer(h1full, D, BF16, l2_body, ag_locfull=(z3loc, z3full))

            # ---- layer 3 + log_softmax ----
            def l3_body(t, mean):
                p3 = po.tile([128, 512], F32, tag="po")
                nc.tensor.matmul(p3[0:Z3W, 0:128], wr3[:, 0:Z3W],
                                 h2T0[:, _ts(t)], start=True, stop=False)
                nc.tensor.matmul(p3[0:Z3W, 0:128], wr3[:, 128:128 + Z3W],
                                 h2T1[:, _ts(t)], start=False, stop=True)
                comb = wp.tile([Z3W, 128], F32, tag="comb")
                nc.vector.tensor_tensor(out=comb[:], in0=mean[0:Z3W, :],
                                        in1=p3[0:Z3W, 0:128],
                                        op=mybir.AluOpType.add)
                comb2 = wp.tile([Z3W, 128], F32, tag="comb2")
                nc.vector.tensor_scalar(out=comb2[:], in0=comb[:],
                                        scalar1=bl3[0:Z3W, 0:1], scalar2=None,
                                        op0=mybir.AluOpType.add)
                ptf = pt.tile([128, 128], F32, tag="pt")
                nc.tensor.transpose(ptf[:, 0:Z3W], comb2[:],
                                    ident32[0:Z3W, 0:Z3W])
                nc.scalar.copy(logits[:, t * NCLS:(t + 1) * NCLS],
                               ptf[:, 0:NCLS])
                scr = wp.tile([128, NCLS], F32, tag="scr")
                nc.scalar.activation(out=scr[:], in_=ptf[:, 0:NCLS],
                                     func=mybir.ActivationFunctionType.Exp,
                                     accum_out=lseA[:, t:t + 1])

            def softmax_batch(t0, t1):
                k = t1 - t0
                lnA = wp.tile([128, 64], F32, tag="lnA")
                nc.scalar.activation(out=lnA[:, 0:k], in_=lseA[:, t0:t1],
                                     func=mybir.ActivationFunctionType.Ln)
                negl = wp.tile([128, 64], F32, tag="negl")
                nc.vector.tensor_scalar(out=negl[:, 0:k], in0=lnA[:, 0:k],
                                        scalar1=-1.0, scalar2=None,
                                        op0=mybir.AluOpType.mult)
                for t in range(t0, t1):
                    fin = wp.tile([128, NCLS], F32, tag="fin")
                    nc.scalar.activation(
                        out=fin[:], in_=logits[:, t * NCLS:(t + 1) * NCLS],
                        func=mybir.ActivationFunctionType.Identity,
                        bias=negl[:, t - t0:t - t0 + 1], scale=1.0)
                    rows = min(128, NLOC - t * 128)
                    nc.sync.dma_start(out=out_d[t * 128:t * 128 + rows, :],
                                      in_=fin[0:rows, :])

            if build_stage >= 3:
                layer(z3full, Z3W, F32, l3_body,
                      epilogue_mid=lambda: softmax_batch(0, 26),
                      epilogue_mid2=lambda: softmax_batch(26, 38),
                      epilogue_end=lambda: softmax_batch(38, NT))

    nc.compile()
    return nc


# ---------------- host side ----------------

def prepare_inputs(x, edge_index, W_map, b_map, Wl1, bl1, Wr1, Wl2, bl2, Wr2,
                   Wl3, bl3, Wr3):
    src = np.asarray(edge_index[0], dtype=np.int64)
    dst = np.asarray(edge_index[1], dtype=np.int64)

    core = dst // NLOC
    dloc = dst - core * NLOC
    t_loc = dloc >> 7
    c_src = src // NLOC
    r_src = src - c_src * NLOC
    p = (r_src >= P0ROWS).astype(np.int64)
    # piece-0 table is AllGathered in two sub-chunks (rows [0,A1) and
    # [A1,3200) per core), so its row layout is [8 x A1][8 x (3200-A1)]
    idx16 = np.where(
        r_src < A1ROWS, c_src * A1ROWS + r_src,
        np.where(r_src < P0ROWS,
                 NCORES * A1ROWS + c_src * A2ROWS + (r_src - A1ROWS),
                 c_src * P1ROWS + (r_src - P0ROWS))).astype(np.int16)

    counts = np.bincount((core * 2 + p) * NT + t_loc,
                         minlength=NCORES * 2 * NT) \
        .reshape(NCORES, 2, NT)
    m = np.maximum(counts.max(axis=0), 128)
    layout = make_layout(m)
    SLTOT = layout["SLTOT"]
    fine_base = layout["fine_base"]       # [2, NT] slot base within core
    dl_base = layout["dl_base"]           # [SLTOT] per-slot 128*tf

    fine = (core * 2 + p) * NT + t_loc
    # secondary sort by gather index: ascending-address rows within each
    # bucket improve HBM row-buffer locality of the edge gather
    order = np.argsort(fine * (1 << 16) + idx16.astype(np.int64), kind="stable")
    gcnt = np.bincount(fine, minlength=NCORES * 2 * NT)
    offs = np.concatenate([[0], np.cumsum(gcnt)])
    rank = np.arange(E) - np.repeat(offs[:-1], gcnt)

    slot = (core[order] * SLTOT + fine_base[p[order], t_loc[order]]
            + rank)
    big_idx = np.zeros(NCORES * SLTOT, dtype=np.int16)
    big_dl = np.tile(dl_base.astype(np.float32) + 999.0, NCORES)
    big_idx[slot] = idx16[order]
    big_dl[slot] = dloc[order].astype(np.float32)
    big_dl -= np.tile(dl_base.astype(np.float32), NCORES)
    big_idx = big_idx.reshape(NCORES, SLTOT)
    big_dl = big_dl.reshape(NCORES, SLTOT)

    # degrees
    cnt = np.bincount(dst, minlength=N).astype(np.float32)
    inv = 1.0 / np.maximum(cnt, 1.0)

    BF = ml_dtypes.bfloat16
    Wmap_pad = np.zeros((F_IN_PAD, 128), np.float32)
    Wmap_pad[0:F_IN] = W_map
    wmap_kt = np.concatenate([Wmap_pad[_ts(k)] for k in range(4)], axis=1)
    Wl3_pad = np.zeros((D2, 128), np.float32)
    Wl3_pad[:, 0:NCLS] = Wl3
    wl3_kt = np.concatenate([Wl3_pad[_ts(k)] for k in range(2)], axis=1)
    Wr3_pad = np.zeros((D2, 128), np.float32)
    Wr3_pad[:, 0:NCLS] = Wr3
    wr3_kt = np.concatenate([Wr3_pad[_ts(k)] for k in range(2)], axis=1)
    bl3_pad = np.zeros((128, 1), np.float32)
    bl3_pad[0:NCLS, 0] = bl3

    shared = {
        "iota": np.ascontiguousarray(
            np.tile(np.arange(256, dtype=np.float32), (128, 1))).astype(BF),
        "ident": np.eye(128, dtype=np.float32).astype(BF),
        "ident32": np.eye(128, dtype=np.float32),
        "wmap": np.ascontiguousarray(wmap_kt).astype(BF),
        "bmap": np.ascontiguousarray(np.asarray(b_map).reshape(128, 1)),
        "wl1": np.ascontiguousarray(Wl1).astype(BF),
        "wr1": np.ascontiguousarray(Wr1).astype(BF),
        "bl1": np.ascontiguousarray(np.asarray(bl1).reshape(128, 1)),
        "wl2": np.ascontiguousarray(Wl2).astype(BF),
        "wr2": np.ascontiguousarray(Wr2).astype(BF),
        "bl2": np.ascontiguousarray(np.asarray(bl2).reshape(2, 128).T),
        "wl3": np.ascontiguousarray(wl3_kt).astype(BF),
        "wr3": np.ascontiguousarray(wr3_kt).astype(BF),
        "bl3": bl3_pad,
    }

    in_maps = []
    for c in range(NCORES):
        xT_pad = np.zeros((F_IN_PAD, NLOC_PAD), np.float32)
        xT_pad[0:F_IN, 0:NLOC] = np.asarray(x)[c * NLOC:(c + 1) * NLOC].T
        idx_arr = np.ascontiguousarray(
            np.tile(big_idx[c].reshape(SLTOT // 16, 16).T, (8, 1)))
        dl_arr = np.ascontiguousarray(
            big_dl[c].reshape(SLTOT // 128, 128).T).astype(BF)
        inv_pad = np.ones(NLOC_PAD, np.float32)
        inv_pad[0:NLOC] = inv[c * NLOC:(c + 1) * NLOC]
        m_ = {
            "xT": xT_pad.astype(BF),
            "idx": idx_arr,
            "dl": dl_arr,
            "invdeg": np.ascontiguousarray(
                np.broadcast_to(inv_pad, (128, NLOC_PAD))),
        }
        m_.update(shared)
        in_maps.append(m_)
    return in_maps, layout


_prog_cache = {}


def kernel(**inputs) -> np.ndarray:
    args = {k: np.asarray(v) for k, v in inputs.items()}
    in_maps, layout = prepare_inputs(
        args["x"], args["edge_index"], args["W_map"], args["b_map"],
        args["Wl1"], args["bl1"], args["Wr1"], args["Wl2"], args["bl2"],
        args["Wr2"], args["Wl3"], args["bl3"], args["Wr3"])

    key = layout["m"].tobytes()
    if key not in _prog_cache:
        _prog_cache[key] = build_program(layout)
    nc = _prog_cache[key]

    trace = os.environ.get("KERNEL_TRACE", "0") == "1"
    kw = {}
    if trace:
        import concourse.bass_utils as bu
        bu.upload_artifacts = lambda t: ""
        kw = dict(trace=True, tmpdir=os.environ.get(
            "KERNEL_TRACE_DIR", "/tmp/kernel_trace"))
    res = run_bass_kernel_spmd(nc, in_maps, list(range(NCORES)), **kw)
    if trace and res.exec_time_ns is not None:
        print(f"HW exec time: {res.exec_time_ns} ns")

    out = np.concatenate([res.results[c]["out"] for c in range(NCORES)], axis=0)
    return out.astype(np.float32)
